# revision 2
# baseline (speedup 1.0000x reference)
"""BindingPocketGNN (3-layer GCN, N=50000, E=800000) on 8 Trainium2 NeuronCores.

v2: the per-subtile indirect_dma_start gather (~1us Q7 descriptor-gen each,
~2550 per core) is replaced by batched gpsimd.dma_gather calls (994ns fixed +
0.34ns/row), gathering thousands of rows per instruction into big SBUF tiles.

Distribution (unchanged): nodes sharded into 8 contiguous ranges (6250/core);
each core aggregates edges whose destination falls in its range, gathering
source features from a replicated node-major table (xs for layer 1,
AllGather-replicated activations for layers 2/3).

dma_gather indices are int16 (max 32767), so edges are split into two classes
by src < 32768; each class gathers from a base-offset view of the table.
Edges are routed host-side to (dst_tile, class, subtile-of-128) slots; per
128-edge subtile a one-hot dst mask is built on DVE and TensorE accumulates
z^T[f, dst_tile] += msg^T @ mask in PSUM. Then y^T = W^T @ (z^T * dinv[dst]),
BN stats AllReduce, fused scale/bias+Relu on ScalarE, transpose to node-major,
dinv-prescale, and AllGather into the next layer's table.
"""
import sys
if "/opt/trn_rl_repo" not in sys.path:
    sys.path.insert(0, "/opt/trn_rl_repo")

import numpy as np

import concourse.bass as bass
import concourse.bacc as bacc
import concourse.mybir as mybir
import concourse.tile as tile
from concourse import bass_utils
from concourse.masks import make_identity

import ml_dtypes
import os

N = 50000
E = 800000
IN, HID = 64, 128
BN_EPS = 1e-5
NCORES = 8
NPC = N // NCORES          # 6250 nodes per core
P = 128
NT = (NPC + P - 1) // P    # 49 dst tiles per core
LAST_D = NPC - (NT - 1) * P  # 106
SPLIT = 32768              # int16 index limit for dma_gather
CT = int(os.environ.get("GCN_CT", "4"))  # dst tiles per gather chunk
# Max subtiles (x128 idxs) per dma_gather call: the SWDGE descriptor ring
# holds ~256 descriptors per SDMA engine; a call emits num_idxs/16+1 per
# engine, so num_idxs must stay well under ~4000.
MAXSUB = int(os.environ.get("GCN_MAXSUB", "15"))
NQ = int(os.environ.get("GCN_NQ", "4"))  # SWDGE queues to stripe gathers over
SKIPAG = os.environ.get("GCN_SKIPAG", "0") == "1"   # timing probe: no AllGather
SKIPAR = os.environ.get("GCN_SKIPAR", "0") == "1"   # timing probe: no stats AllReduce

F32 = mybir.dt.float32
I16 = mybir.dt.int16
I32 = mybir.dt.int32
BF16 = mybir.dt.bfloat16
NP_BF16 = np.dtype(ml_dtypes.bfloat16)

REPS = int(os.environ.get("GCN_REPS", "1"))

Alu = mybir.AluOpType
Act = mybir.ActivationFunctionType

_NC_CACHE = {}


def _plan(S):
    """Static schedule shared by host prep and kernel build.

    S: [NT, 2] subtile counts per (dst tile, src class).
    Returns (suboff[NT,2], calls) where calls is a list of chunks, each a
    list of (klass, g0, nsub) gather calls; subtile global order is
    chunk -> class -> tile -> subtile."""
    S = np.asarray(S)
    suboff = np.zeros((NT, 2), np.int64)
    moff = np.zeros(NT + 1, np.int64)  # per-tile mask-column offsets (tile-major)
    calls = []
    g = 0
    for c0 in range(0, NT, CT):
        tiles = list(range(c0, min(c0 + CT, NT)))
        chunk_calls = []
        for k in (0, 1):
            g0 = g
            for t in tiles:
                suboff[t, k] = g
                g += int(S[t, k])
            # split into ring-capacity-sized calls
            b = g0
            while b < g:
                n = min(MAXSUB, g - b)
                chunk_calls.append((k, b, n))
                b += n
        calls.append((tiles, chunk_calls))
    for t in range(NT):
        moff[t + 1] = moff[t] + int(S[t, 0] + S[t, 1])
    return suboff, moff, calls, g


def _build(S):
    """Build+schedule the SPMD program. S = [NT, 2] subtile counts (shared by
    all 8 cores; per-core shortfalls are padded with index-0 gathers that the
    dst mask kills)."""
    S = np.asarray(S)
    suboff, moff, calls, T = _plan(S)
    TOTC = T * 8  # idx cols (int16, 16-partition wrap)

    nc = bacc.Bacc("TRN2", target_bir_lowering=False, debug=False, num_devices=NCORES,
                   num_swdge_queues=NQ)

    # ---- I/O ----
    xs = nc.dram_tensor("xs", [N, HID], BF16, kind="ExternalInput")
    iota_d = nc.dram_tensor("iotam", [P, P], BF16, kind="ExternalInput")
    ident_d = nc.dram_tensor("identm", [P, P], F32, kind="ExternalInput")
    gidx_d = nc.dram_tensor("gidx", [P, TOTC], I16, kind="ExternalInput")
    dloc_d = nc.dram_tensor("dloc", [P, T], BF16, kind="ExternalInput")
    dinv_d = nc.dram_tensor("dinv_sl", [P, NT], F32, kind="ExternalInput")
    W_d = [
        nc.dram_tensor("W1", [IN, HID], F32, kind="ExternalInput"),
        nc.dram_tensor("W2", [HID, HID], F32, kind="ExternalInput"),
        nc.dram_tensor("W3", [HID, HID], F32, kind="ExternalInput"),
    ]
    fcW_d = nc.dram_tensor("fcW", [HID, 1], F32, kind="ExternalInput")
    g_d = [nc.dram_tensor("g1", [HID, 1], F32, kind="ExternalInput"),
           nc.dram_tensor("g2", [HID, 1], F32, kind="ExternalInput")]
    bt_d = [nc.dram_tensor("bt1", [HID, 1], F32, kind="ExternalInput"),
            nc.dram_tensor("bt2", [HID, 1], F32, kind="ExternalInput")]
    b3_d = nc.dram_tensor("b3", [HID, 1], F32, kind="ExternalInput")
    outv = nc.dram_tensor("outv", [1, NPC], F32, kind="ExternalOutput")

    with tile.TileContext(nc) as tc:
        with (
            tc.tile_pool(name="meta", bufs=1) as meta,
            tc.tile_pool(name="gatp", bufs=8) as gatp,
            tc.tile_pool(name="maskp", bufs=4) as maskp,
            tc.tile_pool(name="zsp", bufs=3) as zsp,
            tc.tile_pool(name="actp", bufs=3) as actp,
            tc.tile_pool(name="hp", bufs=3) as hp,
            tc.tile_pool(name="sqp", bufs=2) as sqp,
            tc.tile_pool(name="zps_p", bufs=2, space="PSUM") as zps_p,
            tc.tile_pool(name="yps_p", bufs=2, space="PSUM") as yps_p,
            tc.tile_pool(name="trps_p", bufs=2, space="PSUM") as trps_p,
            tc.tile_pool(name="fcps_p", bufs=1, space="PSUM") as fcps_p,
            tc.tile_pool(name="dram", bufs=1, space="DRAM") as dram,
        ):
            # ---- resident metadata ----
            gidx_sb = meta.tile([P, TOTC], I16)
            nc.sync.dma_start(gidx_sb[:], gidx_d[:])
            dloc_sb = meta.tile([P, T], BF16)
            nc.sync.dma_start(dloc_sb[:], dloc_d[:])
            dinv_sl = meta.tile([P, NT], F32)
            nc.sync.dma_start(dinv_sl[:], dinv_d[:])
            W_sb = []
            for l in range(3):
                fi = IN if l == 0 else HID
                w = meta.tile([fi, HID], F32, name=f"W{l}_sb")
                nc.sync.dma_start(w[:], W_d[l][:])
                W_sb.append(w)
            fcW_sb = meta.tile([HID, 1], F32)
            nc.sync.dma_start(fcW_sb[:], fcW_d[:])
            g_sb, bt_sb = [], []
            for l in range(2):
                gg = meta.tile([HID, 1], F32, name=f"g{l}_sb")
                nc.sync.dma_start(gg[:], g_d[l][:])
                g_sb.append(gg)
                bb = meta.tile([HID, 1], F32, name=f"bt{l}_sb")
                nc.sync.dma_start(bb[:], bt_d[l][:])
                bt_sb.append(bb)
            b3_sb = meta.tile([HID, 1], F32)
            nc.sync.dma_start(b3_sb[:], b3_d[:])
            eps_sb = meta.tile([P, 1], F32)
            nc.vector.memset(eps_sb[:], BN_EPS)

            ident = meta.tile([P, P], F32)
            nc.sync.dma_start(ident[:], ident_d[:])
            iota_t = meta.tile([P, P], BF16)
            nc.sync.dma_start(iota_t[:], iota_d[:])

            # dinv broadcast rows: dinv_bc[:, t*128+j] = dinv of node t*128+j
            dinv_bc = meta.tile([P, NT * P], F32)
            for t in range(NT):
                tr = trps_p.tile([P, P], F32, tag="tr")
                nc.tensor.transpose(tr[:], dinv_sl[:, t:t + 1].to_broadcast([P, P]), ident[:])
                nc.vector.tensor_copy(dinv_bc[:, t * P:(t + 1) * P], tr[:])

            ystore = meta.tile([P, NT * P], F32)
            sums = meta.tile([P, NT], F32)
            sumsq = meta.tile([P, NT], F32)
            out_store = meta.tile([1, NPC], F32)

            # internal DRAM for collectives (fresh per rep: Shared tensors allow one writer)
            def mk_coll(rep):
                tab_in = [dram.tile([NPC, HID], BF16, name=f"tab{l}_in_r{rep}") for l in (1, 2)]
                tab_out = [dram.tile([N, HID], BF16, name=f"tab{l}_out_r{rep}", addr_space="Shared")
                           for l in (1, 2)]
                st_in = [dram.tile([P, 2], F32, name=f"st{l}_in_r{rep}") for l in (0, 1)]
                st_out = [dram.tile([P, 2], F32, name=f"st{l}_out_r{rep}", addr_space="Shared")
                          for l in (0, 1)]
                return tab_in, tab_out, st_in, st_out

            for _rep in range(REPS):
              tab_in, tab_out, st_in, st_out = mk_coll(_rep)
              for l in range(3):
                 f_in = IN if l == 0 else HID
                 table = xs if l == 0 else tab_out[l - 1]
                 tab_view = [table[0:SPLIT, :], table[SPLIT:N, :]]
                 # ---- aggregation + weight matmul, chunk-pipelined ----
                 qi = 0
                 for tiles, chunk_calls in calls:
                     gat = {}  # subtile g -> (tile, g0) via range lookup
                     for (k, g0, nsub) in chunk_calls:
                         gt = gatp.tile([P, nsub * HID], BF16, tag="gat")
                         nc.gpsimd.dma_gather(
                             out_ap=gt[:].rearrange("p (s e) -> p s e", e=HID),
                             in_ap=tab_view[k],
                             idxs_ap=gidx_sb[:, g0 * 8:(g0 + nsub) * 8],
                             num_idxs=nsub * P,
                             num_idxs_reg=nsub * P,
                             elem_size=HID,
                             queue_num=qi % NQ,
                         )
                         qi += 1
                         for g in range(g0, g0 + nsub):
                             gat[g] = (gt, g0)
                     for t in tiles:
                         d_hi = LAST_D if t == NT - 1 else P
                         total = int(S[t, 0] + S[t, 1])
                         m0 = int(moff[t])
                         mb = maskp.tile([P, total * P], BF16, tag="mask")
                         nc.vector.tensor_tensor(
                             out=mb[:].rearrange("p (s e) -> p s e", e=P),
                             in0=iota_t[:].rearrange("p e -> p () e").to_broadcast([P, total, P]),
                             in1=dloc_sb[:, m0:m0 + total].rearrange("p s -> p s ()").to_broadcast([P, total, P]),
                             op=Alu.is_equal,
                         )
                         zps = zps_p.tile([P, P], F32, tag="zps")
                         mm = 0
                         for k in (0, 1):
                             for s in range(int(S[t, k])):
                                 g = int(suboff[t, k]) + s
                                 gt, g0 = gat[g]
                                 b = (g - g0) * HID
                                 nc.tensor.matmul(zps[:f_in, :], lhsT=gt[:, b:b + f_in],
                                                  rhs=mb[:, mm * P:(mm + 1) * P],
                                                  start=(mm == 0), stop=(mm == total - 1))
                                 mm += 1
                         zs = zsp.tile([P, P], F32, tag="zs")
                         nc.vector.tensor_tensor(
                             out=zs[:f_in, :], in0=zps[:f_in],
                             in1=dinv_bc[:f_in, t * P:(t + 1) * P], op=Alu.mult,
                         )
                         yps = yps_p.tile([P, P], F32, tag="yps")
                         nc.tensor.matmul(yps[:], lhsT=W_sb[l][:], rhs=zs[:f_in, :],
                                          start=True, stop=True)
                         if l < 2:
                             nc.scalar.activation(
                                 out=ystore[:, t * P:t * P + d_hi], in_=yps[:, :d_hi],
                                 func=Act.Copy, accum_out=sums[:, t:t + 1],
                             )
                             sq = sqp.tile([P, P], F32, tag="sq")
                             nc.scalar.activation(
                                 out=sq[:, :d_hi], in_=yps[:, :d_hi],
                                 func=Act.Square, accum_out=sumsq[:, t:t + 1],
                             )
                         else:
                             act3 = actp.tile([P, P], F32, tag="act")
                             nc.scalar.activation(out=act3[:, :d_hi], in_=yps[:, :d_hi],
                                                  func=Act.Relu, bias=b3_sb[:], scale=1.0)
                             fcp = fcps_p.tile([1, P], F32, tag="fcp")
                             nc.tensor.matmul(fcp[:1, :d_hi], lhsT=fcW_sb[:], rhs=act3[:, :d_hi],
                                              start=True, stop=True)
                             nc.vector.tensor_copy(out_store[:1, t * P:t * P + d_hi], fcp[:1, :d_hi])

                 if l < 2:
                     # ---- BN stats allreduce + coefficients ----
                     stats = meta.tile([P, 2], F32, name=f"stats{l}_r{_rep}")
                     nc.vector.tensor_reduce(stats[:, 0:1], sums[:], axis=mybir.AxisListType.X, op=Alu.add)
                     nc.vector.tensor_reduce(stats[:, 1:2], sumsq[:], axis=mybir.AxisListType.X, op=Alu.add)
                     nc.sync.dma_start(st_in[l][:], stats[:])
                     if not SKIPAR:
                         nc.gpsimd.collective_compute(
                             "AllReduce", Alu.add, replica_groups=[list(range(NCORES))],
                             ins=[st_in[l][:]], outs=[st_out[l][:]],
                         )
                     tot = meta.tile([P, 2], F32, name=f"tot{l}_r{_rep}")
                     nc.sync.dma_start(tot[:], st_out[l][:] if not SKIPAR else st_in[l][:])
                     cf = meta.tile([P, 6], F32, name=f"cf{l}_r{_rep}")  # mean ex2 var std A B
                     nc.vector.tensor_scalar_mul(cf[:, 0:1], tot[:, 0:1], 1.0 / N)
                     nc.vector.tensor_scalar_mul(cf[:, 1:2], tot[:, 1:2], 1.0 / N)
                     nc.vector.tensor_tensor(out=cf[:, 2:3], in0=cf[:, 0:1], in1=cf[:, 0:1], op=Alu.mult)
                     nc.vector.tensor_tensor(out=cf[:, 2:3], in0=cf[:, 1:2], in1=cf[:, 2:3], op=Alu.subtract)
                     nc.scalar.activation(out=cf[:, 3:4], in_=cf[:, 2:3], func=Act.Sqrt, bias=eps_sb[:], scale=1.0)
                     nc.vector.reciprocal(cf[:, 4:5], cf[:, 3:4])
                     A = meta.tile([P, 1], F32, name=f"A{l}_r{_rep}")
                     B = meta.tile([P, 1], F32, name=f"B{l}_r{_rep}")
                     nc.vector.tensor_tensor(out=A[:], in0=g_sb[l][:], in1=cf[:, 4:5], op=Alu.mult)
                     nc.vector.tensor_tensor(out=cf[:, 5:6], in0=cf[:, 0:1], in1=A[:], op=Alu.mult)
                     nc.vector.tensor_tensor(out=B[:], in0=bt_sb[l][:], in1=cf[:, 5:6], op=Alu.subtract)

                     # ---- epilogue: act, transpose to node-major, store table slice ----
                     for t in range(NT):
                         d_hi = LAST_D if t == NT - 1 else P
                         act = actp.tile([P, P], F32, tag="act")
                         nc.scalar.activation(out=act[:, :d_hi], in_=ystore[:, t * P:t * P + d_hi],
                                              func=Act.Relu, bias=B[:], scale=A[:])
                         tr = trps_p.tile([P, P], F32, tag="tr")
                         nc.tensor.transpose(tr[:d_hi, :], act[:, :d_hi], ident[:])
                         h = hp.tile([P, HID], BF16, tag="h")
                         nc.vector.tensor_scalar_mul(h[:d_hi, :], tr[:d_hi, :], dinv_sl[:d_hi, t:t + 1])
                         nc.sync.dma_start(tab_in[l][t * P:t * P + d_hi, :], h[:d_hi, :])
                     if SKIPAG:
                         nc.sync.dma_start(tab_out[l][0:NPC, :], tab_in[l][:])
                     else:
                         nc.gpsimd.collective_compute(
                             "AllGather", Alu.bypass, replica_groups=[list(range(NCORES))],
                             ins=[tab_in[l][:]], outs=[tab_out[l][:]],
                         )

            nc.sync.dma_start(outv[:], out_store[:])

    nc.compile()
    return nc


def _prep(inputs):
    x = np.asarray(inputs["x"], np.float32)
    ei = np.asarray(inputs["edge_index"], np.int64)
    loops = np.arange(N, dtype=np.int64)
    src = np.concatenate([ei[0], loops])
    dst = np.concatenate([ei[1], loops])
    deg = np.bincount(dst, minlength=N).astype(np.float32)
    dinv = (1.0 / np.sqrt(deg)).astype(np.float32)
    xs = np.zeros((N, HID), NP_BF16)
    xs[:, :IN] = (x * dinv[:, None]).astype(NP_BF16)

    core = dst // NPC
    rem = dst - core * NPC
    tidx = rem >> 7
    loc = (rem & 127).astype(np.float32)
    klass = (src >= SPLIT).astype(np.int64)

    gk = (core * NT + tidx) * 2 + klass
    cnt = np.bincount(gk, minlength=NCORES * NT * 2).reshape(NCORES, NT, 2)
    S = np.ceil(cnt.max(axis=0) / P).astype(np.int64)  # [NT, 2]
    suboff, moff, calls, T = _plan(S)

    order = np.lexsort((klass, tidx, core))
    src_s = src[order]
    core_s = core[order]
    tidx_s = tidx[order]
    klass_s = klass[order]
    loc_s = loc[order]
    gk_s = gk[order]

    starts = np.zeros(NCORES * NT * 2, np.int64)
    starts[1:] = np.cumsum(cnt.reshape(-1))[:-1]
    pos = np.arange(len(src_s)) - starts[gk_s]
    sub = pos >> 7
    lane = (pos & 127).astype(np.int64)
    gsub = suboff[tidx_s, klass_s] + sub

    val = (src_s - klass_s * SPLIT).astype(np.int16)
    colv = gsub * 8 + (lane >> 4)
    rowv = lane & 15

    gidx = np.zeros((NCORES, 16, T * 8), np.int16)
    gidx[core_s, rowv, colv] = val
    gidx = np.tile(gidx, (1, 8, 1))  # replicate across the 8 Q7 core groups

    # mask columns are tile-major: per tile, class-0 then class-1 subtiles
    mcol = moff[tidx_s] + (gsub - suboff[tidx_s, klass_s]) + klass_s * S[tidx_s, 0]
    dloc = np.full((NCORES, P, T), 1000.0, NP_BF16)
    dloc[core_s, lane, mcol] = loc_s

    dv = dinv.reshape(NCORES, NPC)
    dinv_pad = np.zeros((NCORES, NT * P), np.float32)
    dinv_pad[:, :NPC] = dv
    dinv_sl = dinv_pad.reshape(NCORES, NT, P).transpose(0, 2, 1).copy()  # [c, P, NT]

    com = {
        "xs": np.ascontiguousarray(xs),
        "iotam": np.broadcast_to(np.arange(P, dtype=np.float32), (P, P)).astype(NP_BF16),
        "identm": np.eye(P, dtype=np.float32),
        "W1": np.asarray(inputs["W1"], np.float32),
        "W2": np.asarray(inputs["W2"], np.float32),
        "W3": np.asarray(inputs["W3"], np.float32),
        "fcW": np.asarray(inputs["fcW"], np.float32).reshape(HID, 1),
        "g1": np.asarray(inputs["g1"], np.float32).reshape(HID, 1),
        "g2": np.asarray(inputs["g2"], np.float32).reshape(HID, 1),
        "bt1": np.asarray(inputs["bt1"], np.float32).reshape(HID, 1),
        "bt2": np.asarray(inputs["bt2"], np.float32).reshape(HID, 1),
        "b3": np.asarray(inputs["b3"], np.float32).reshape(HID, 1),
    }
    in_maps = []
    for c in range(NCORES):
        m = dict(com)
        m["gidx"] = np.ascontiguousarray(gidx[c])
        m["dloc"] = np.ascontiguousarray(dloc[c])
        m["dinv_sl"] = np.ascontiguousarray(dinv_sl[c])
        in_maps.append(m)
    return in_maps, tuple(map(tuple, S.tolist()))


def _get_nc(S):
    key = (S, REPS, CT, MAXSUB, NQ, SKIPAG, SKIPAR)
    if key not in _NC_CACHE:
        _NC_CACHE[key] = _build(np.asarray(S))
    return _NC_CACHE[key]


class _Exec:
    """jit-once / device_put-once executor mirroring bass2jax.run_bass_via_pjrt."""

    def __init__(self, nc, in_maps):
        import jax
        from jax.sharding import Mesh, PartitionSpec
        from jax.experimental.shard_map import shard_map
        from concourse import bass2jax
        bass2jax.install_neuronx_cc_hook()
        n_cores = NCORES
        part_name = nc.partition_id_tensor.name if nc.partition_id_tensor else None
        in_names, out_names, out_avals, zero_outs = [], [], [], []
        for alloc in nc.m.functions[0].allocations:
            if not isinstance(alloc, mybir.MemoryLocationSet):
                continue
            name = alloc.memorylocations[0].name
            if alloc.kind == "ExternalInput":
                if name != part_name:
                    in_names.append(name)
            elif alloc.kind == "ExternalOutput":
                out_names.append(name)
                shape = tuple(alloc.tensor_shape)
                dtype = mybir.dt.np(alloc.dtype)
                out_avals.append(jax.core.ShapedArray(shape, dtype))
                zero_outs.append(np.zeros(shape, dtype))
        n_params = len(in_names)
        all_names = in_names + out_names
        if part_name is not None:
            all_names = all_names + [part_name]
        self.out_names, self.out_avals, self.n_cores = out_names, out_avals, n_cores

        def _body(*args):
            operands = list(args)
            if part_name is not None:
                operands.append(bass2jax.partition_id_tensor())
            outs = bass2jax._bass_exec_p.bind(
                *operands,
                out_avals=tuple(out_avals),
                in_names=tuple(all_names),
                out_names=tuple(out_names),
                lowering_input_output_aliases=(),
                sim_require_finite=True,
                sim_require_nnan=True,
                nc=nc,
            )
            return tuple(outs)

        devices = jax.devices()[:n_cores]
        mesh = Mesh(np.asarray(devices), ("core",))
        in_specs = (PartitionSpec("core"),) * (n_params + len(out_names))
        out_specs = (PartitionSpec("core"),) * len(out_names)
        self.fn = jax.jit(
            shard_map(_body, mesh=mesh, in_specs=in_specs, out_specs=out_specs,
                      check_rep=False),
            keep_unused=True,
        )
        concat_in = [
            np.concatenate([np.asarray(in_maps[c][k]) for c in range(n_cores)], axis=0)
            for k in in_names
        ]
        concat_zeros = [
            np.zeros((n_cores * z.shape[0], *z.shape[1:]), z.dtype) for z in zero_outs
        ]
        sh = jax.sharding.NamedSharding(mesh, PartitionSpec("core"))
        self.dev_in = [jax.device_put(a, sh) for a in concat_in] + \
                      [jax.device_put(a, sh) for a in concat_zeros]
        for a in self.dev_in:
            a.block_until_ready()

    def run(self):
        outs = self.fn(*self.dev_in)
        for o in outs:
            o.block_until_ready()
        return outs

    def results(self):
        outs = self.run()
        res = [dict() for _ in range(self.n_cores)]
        for i, name in enumerate(self.out_names):
            arr = np.asarray(outs[i]).reshape(self.n_cores, *self.out_avals[i].shape)
            for c in range(self.n_cores):
                res[c][name] = arr[c]
        return res


_EXEC_CACHE = {}


def _get_exec(in_maps, S):
    key = (S, REPS, CT, MAXSUB, NQ, SKIPAG, SKIPAR)
    if key not in _EXEC_CACHE:
        _EXEC_CACHE[key] = _Exec(_get_nc(S), in_maps)
    return _EXEC_CACHE[key]


def _run(in_maps, S):
    nc = _get_nc(S)
    r = bass_utils.run_bass_kernel_spmd(nc, in_maps, core_ids=list(range(NCORES)), trace=False)
    return r


def kernel(**inputs):
    in_maps, S = _prep(inputs)
    r = _run(in_maps, S)
    out = np.concatenate([r.results[c]["outv"].reshape(-1) for c in range(NCORES)])
    fcb = np.asarray(inputs["fcb"], np.float32).reshape(-1)
    out = (out + fcb[0]).astype(np.float32)[:, None]
    # numerically stable sigmoid in fp32
    sig = np.empty_like(out)
    pos = out >= 0
    sig[pos] = 1.0 / (1.0 + np.exp(-out[pos], dtype=np.float32))
    ex = np.exp(out[~pos], dtype=np.float32)
    sig[~pos] = ex / (1.0 + ex)
    return out, sig
